# revision 1
# baseline (speedup 1.0000x reference)
"""Self-contained TRN2 Bass kernel for GCNConv + PReLU (nn_Encoder_11536282157710).

kernel(**inputs) takes the FULL inputs (x [100000,128] f32,
edge_index [2,1600000] i64, W [128,64] f32, b [64] f32, prelu_w [64] f32)
and returns the FULL output [100000,64] f32, computed on 8 TRN2 NeuronCores.

Math:  deg[v] = in_degree(v) + 1;  dinv = deg**-0.5
       agg[d] = sum_{(s,d)} dinv[s]*dinv[d]*x[s]   (self loop = edge (d,d))
       out[d] = prelu(agg[d] @ W + b)

Distribution (one Bass program per core; loop structure is data-dependent):
  - destination nodes partitioned 8 ways (12500 rows/core)
  - per core: dst windows of 1024 slots; per (window, 32768-row src bucket)
    the edges are sorted by dst and 128-padded (pads: idx=0, dstslot=-1)
  - dma_gather pulls x[src] rows (fp16, 256B) per run segment; self-loop rows
    arrive via plain strided HWDGE DMA (no per-row descriptors)
  - one-hot matrices M[p,piece,t] = (dstslot[p,piece]==t) * dinv[s]*dinv[d]
    are built in 2 batched DVE ops per <=128-piece group over 16-wide
    aligned slot stripes
  - per 128-message block, one fp16 matmul accumulates
    psum[:, 16*g0:16*(g0+ng)] += x_block^T @ M[:, pieces, :]  (PSUM fp32)
  - per window: evac PSUM, out_chunk = aggT_chunk^T @ W (fp32), bias via
    PSUM preload, prelu(v) = relu(v) - prelu_w * relu(-v), DMA out
"""

import sys
sys.path.insert(0, '/opt/trn_rl_repo')

import numpy as np
from concurrent.futures import ThreadPoolExecutor

from concourse.bass import AP
import concourse.bacc as bacc
import concourse.mybir as mybir
import concourse.tile as tile
from concourse.bass_utils import run_bass_kernel_spmd

F32 = mybir.dt.float32
F32R = mybir.dt.float32r
F16 = mybir.dt.float16
I16 = mybir.dt.int16

BUCKET = 32768
WSLOTS = 1024
WS = 16          # stripe width
SEGC = 16        # msg-tile columns per gather segment (2048 msgs)
HID = 64
P = 128


def _round_up(x, m):
    return (x + m - 1) // m * m


def build_core_metadata(src, dst, deg, n_nodes, n_cores, core):
    """Integer-only host preprocessing for one core."""
    n_per = n_nodes // n_cores
    lo, hi = core * n_per, (core + 1) * n_per
    m = (dst >= lo) & (dst < hi)
    s_c = src[m]
    dloc = dst[m] - lo
    win = dloc // WSLOTS
    bucket = s_c // BUCKET
    order = np.lexsort((dloc, bucket, win))
    s_c, dloc, win, bucket = s_c[order], dloc[order], win[order], bucket[order]
    deg_s = deg[s_c]
    deg_d = deg[lo + dloc]

    n_win = (n_per + WSLOTS - 1) // WSLOTS
    n_buckets = (n_nodes + BUCKET - 1) // BUCKET
    counts = np.bincount(win * n_buckets + bucket, minlength=n_win * n_buckets)

    idx16_l, dstst_l, dppc_l = [], [], []
    windows = []
    goff = 0      # gather stream position (128-aligned)
    soff = 0      # stream offset into sorted edge arrays
    qrot = 0
    max_run_cols = 1
    for w in range(n_win):
        runs, blocks = [], []
        idx_col_base = goff // 128
        # pending per-window piece columns are appended to dstst_l in order
        piece_col0 = sum(a.shape[1] for a in dstst_l)

        def add_blocks(run_i, dstslot_pad, degprod_pad, run_of=None):
            """dstslot_pad: [Lpad] window-local dst slots (-1 = pad)."""
            nonlocal dstst_l, dppc_l
            L = len(dstslot_pad)
            for b in range(L // 128):
                sl = slice(b * 128, (b + 1) * 128)
                dv = dstslot_pad[sl]
                dp = degprod_pad[sl]
                valid = dv >= 0
                if not valid.any():
                    continue
                g0 = int(dv[valid].min()) // WS
                g1 = int(dv[valid].max()) // WS
                chunks = []
                g = g0
                gpb = 512 // WS  # groups per PSUM bank; a matmul can't cross banks
                while g <= g1:
                    ng = min(g1 - g + 1, gpb - (g % gpb))
                    pc = sum(a.shape[1] for a in dstst_l)
                    dcols = np.full((128, ng), -1.0, dtype=np.float32)
                    pcols = np.ones((128, ng), dtype=np.float32)
                    for gi in range(ng):
                        mm = valid & (dv // WS == g + gi)
                        dcols[mm, gi] = dv[mm] - (g + gi) * WS
                        pcols[:, gi] = dp
                    dstst_l.append(dcols)
                    dppc_l.append(pcols)
                    chunks.append((g, ng, pc))
                    g += ng
                ri, bc = run_of(b) if run_of else (run_i, b)
                blocks.append(dict(run_i=ri, bcol=bc, chunks=chunks))

        for b in range(n_buckets):
            n = int(counts[w * n_buckets + b])
            if n == 0:
                continue
            L = _round_up(n, 128)
            sl = slice(soff, soff + n)
            idx = np.zeros(L, dtype=np.int16)
            idx[:n] = (s_c[sl] - b * BUCKET).astype(np.int16)
            dv = np.full(L, -1.0, dtype=np.float32)
            dv[:n] = (dloc[sl] - w * WSLOTS).astype(np.float32)
            dp = np.ones(L, dtype=np.float32)
            dp[:n] = (deg_s[sl] * deg_d[sl]).astype(np.float32)
            idx16_l.append(idx)
            run_base = len(runs)
            ncols_all = L // 128
            for s0 in range(0, ncols_all, SEGC):
                sc = min(SEGC, ncols_all - s0)
                runs.append(dict(kind="gather", bucket=b,
                                 idx_col0=goff // 128 + s0, n_cols=sc,
                                 queue=qrot % 4))
                qrot += 1
            add_blocks(None, dv, dp,
                       run_of=lambda blk_col: (run_base + blk_col // SEGC,
                                               blk_col % SEGC))
            max_run_cols = max(max_run_cols, min(SEGC, ncols_all))
            goff += L
            soff += n
        # self-loop run: rows w*WSLOTS .. min(+WSLOTS, n_per)
        r0 = w * WSLOTS
        cnt = min(WSLOTS, n_per - r0)
        Ls = _round_up(cnt, 128)
        dv = np.full(Ls, -1.0, dtype=np.float32)
        dv[:cnt] = (np.arange(r0, r0 + cnt) - r0).astype(np.float32)
        degw = deg[lo + r0:lo + r0 + cnt].astype(np.float32)
        dp = np.ones(Ls, dtype=np.float32)
        dp[:cnt] = degw * degw
        run_base = len(runs)
        for s0 in range(0, Ls // 128, SEGC):
            sc = min(SEGC, Ls // 128 - s0)
            runs.append(dict(kind="self", row0=r0 + s0 * 128,
                             rows=min(sc * 128, cnt - s0 * 128), n_cols=sc))
        add_blocks(None, dv, dp,
                   run_of=lambda blk_col: (run_base + blk_col // SEGC,
                                           blk_col % SEGC))
        max_run_cols = max(max_run_cols, min(SEGC, Ls // 128))

        n_pieces_w = sum(a.shape[1] for a in dstst_l) - piece_col0
        # split the window's pieces into M-groups of <=128 piece columns,
        # never splitting a chunk
        mgroups = []  # (local piece col0, count)
        cur0, cur = 0, 0
        for blk in blocks:
            new_chunks = []
            for (g0, ng, pc) in blk["chunks"]:
                j = pc - piece_col0
                if cur > 0 and (j + ng - cur0) > 128:
                    mgroups.append((cur0, cur))
                    cur0, cur = j, 0
                cur = j + ng - cur0
                new_chunks.append((g0, ng, len(mgroups), j - cur0))
            blk["chunks"] = new_chunks
        if cur > 0:
            mgroups.append((cur0, cur))
        windows.append(dict(runs=runs, blocks=blocks, piece_col0=piece_col0,
                            n_pieces=n_pieces_w, idx_col_base=idx_col_base,
                            n_idx_cols=goff // 128 - idx_col_base,
                            mgroups=mgroups))
    assert soff == len(s_c)

    # wrap per-position columns: position i of a block -> partition i%128
    def wrap_pos(cols):
        # each entry currently [Lpad]-flat per-position for dstst built above?
        return cols

    gtot = goff
    idx_flat = np.concatenate(idx16_l) if idx16_l else np.zeros(0, np.int16)
    assert len(idx_flat) == gtot
    idx_w = np.tile(idx_flat.reshape(gtot // 16, 16).T, (8, 1)).copy() if gtot else np.zeros((128, 0), np.int16)
    dstst = np.concatenate(dstst_l, axis=1) if dstst_l else np.zeros((128, 0), np.float32)
    dppc = np.concatenate(dppc_l, axis=1) if dppc_l else np.zeros((128, 0), np.float32)

    return dict(core=core, n_per=n_per, n_win=n_win, windows=windows,
                gtot=gtot, n_pieces=dstst.shape[1], max_run_cols=max_run_cols,
                idx=idx_w, dstst=dstst, dppc=dppc)


def _pad_arrays(meta):
    """Sanity + final array shapes."""
    return meta


def build_core_kernel(meta, in_c=128, msg_bufs=16):
    nc = bacc.Bacc("TRN2", target_bir_lowering=False, debug=False, num_swdge_queues=4)
    n_per = meta["n_per"]
    N = meta["n_nodes"]
    n_pieces = max(meta["n_pieces"], 1)
    gcols = max(meta["gtot"] // 128, 1)

    x = nc.dram_tensor("x", [N, in_c], F16, kind="ExternalInput")
    w_in = nc.dram_tensor("w_in", [in_c, HID], F32, kind="ExternalInput")
    b_bc = nc.dram_tensor("b_bc", [P, HID], F32, kind="ExternalInput")
    pw_bc = nc.dram_tensor("pw_bc", [P, HID], F32, kind="ExternalInput")
    idx_in = nc.dram_tensor("idx", [P, gcols * 8], I16, kind="ExternalInput")
    dstst_in = nc.dram_tensor("dstst", [P, n_pieces], F16, kind="ExternalInput")
    dppc_in = nc.dram_tensor("dppc", [P, n_pieces], F32, kind="ExternalInput")
    out = nc.dram_tensor("out", [n_per, HID], F32, kind="ExternalOutput")

    max_w_pieces = max(w["n_pieces"] for w in meta["windows"])

    with tile.TileContext(nc) as tc:
        with (
            tc.tile_pool(name="const", bufs=1) as cpool,
            tc.tile_pool(name="wmeta", bufs=3) as wpool,
            tc.tile_pool(name="msgs", bufs=msg_bufs) as mpool,
            tc.tile_pool(name="mt", bufs=4) as mtpool,
            tc.tile_pool(name="agg", bufs=2) as apool,
            tc.tile_pool(name="fin", bufs=2) as fpool,
            tc.tile_pool(name="psum", bufs=2, space="PSUM") as ppool,
            tc.tile_pool(name="psum_o", bufs=2, space="PSUM") as popool,
        ):
            iota_t = cpool.tile([P, WS], F16)
            nc.gpsimd.iota(iota_t[:], pattern=[[1, WS]], base=0, channel_multiplier=0,
                           allow_small_or_imprecise_dtypes=True)
            wmat = cpool.tile([in_c, HID], F32)
            nc.sync.dma_start(out=wmat[:], in_=w_in[:])
            b_t = cpool.tile([P, HID], F32)
            nc.sync.dma_start(out=b_t[:], in_=b_bc[:])
            pw_t = cpool.tile([P, HID], F32)
            nc.sync.dma_start(out=pw_t[:], in_=pw_bc[:])

            max_icols = max(w["n_idx_cols"] for w in meta["windows"])
            max_wp = max(w["n_pieces"] for w in meta["windows"])
            max_mg = max((n for w in meta["windows"] for _, n in w["mgroups"]), default=1)

            for w, win in enumerate(meta["windows"]):
                pc0 = win["piece_col0"]
                npw = win["n_pieces"]
                # stage this window's metadata
                idx_t = wpool.tile([P, max(max_icols, 1) * 8], I16, tag="widx")
                if win["n_idx_cols"]:
                    nc.sync.dma_start(
                        out=idx_t[:, :win["n_idx_cols"] * 8],
                        in_=idx_in[:, win["idx_col_base"] * 8:
                                   (win["idx_col_base"] + win["n_idx_cols"]) * 8])
                dstst_t = wpool.tile([P, max_wp], F16, tag="wdst")
                nc.sync.dma_start(out=dstst_t[:, :npw], in_=dstst_in[:, pc0:pc0 + npw])
                normf_t = wpool.tile([P, max_wp], F32, tag="wnormf")
                nc.sync.dma_start(out=normf_t[:, :npw], in_=dppc_in[:, pc0:pc0 + npw])
                nc.vector.reciprocal(normf_t[:, :npw], normf_t[:, :npw])
                nc.scalar.sqrt(normf_t[:, :npw], normf_t[:, :npw])
                norm_t = wpool.tile([P, max_wp], F16, tag="wnorm")
                nc.scalar.activation(norm_t[:, :npw], normf_t[:, :npw],
                                     mybir.ActivationFunctionType.Copy)

                # batched M build per m-group
                m_tiles = []
                for (j0, cnt) in win["mgroups"]:
                    m_t = mtpool.tile([P, max_mg, WS], F16, tag="m")
                    it_b = AP(iota_t[:].tensor, iota_t[:].offset,
                              [iota_t[:].ap[0], [0, cnt], [1, WS]])
                    ds = dstst_t[:, j0:j0 + cnt]
                    ds_b = AP(ds.tensor, ds.offset, [ds.ap[0], [1, cnt], [0, WS]])
                    nm = norm_t[:, j0:j0 + cnt]
                    nm_b = AP(nm.tensor, nm.offset, [nm.ap[0], [1, cnt], [0, WS]])
                    nc.vector.tensor_tensor(out=m_t[:, :cnt, :], in0=it_b, in1=ds_b,
                                            op=mybir.AluOpType.is_equal)
                    nc.vector.tensor_tensor(out=m_t[:, :cnt, :], in0=m_t[:, :cnt, :],
                                            in1=nm_b, op=mybir.AluOpType.mult)
                    m_tiles.append(m_t)

                psum_w = ppool.tile([P, WSLOTS], F32, tag="pw")
                nc.vector.memset(psum_w[:], 0.0)

                run_tiles = []
                for r in win["runs"]:
                    rt = mpool.tile([P, meta["max_run_cols"], in_c], F16, tag="msg")
                    if r["kind"] == "gather":
                        L = r["n_cols"] * 128
                        bkt = r["bucket"]
                        ic0 = r["idx_col0"] - win["idx_col_base"]
                        nc.gpsimd.dma_gather(
                            out_ap=rt[:, :r["n_cols"], :],
                            in_ap=x[bkt * BUCKET:min((bkt + 1) * BUCKET, N), :],
                            idxs_ap=idx_t[:, ic0 * 8:(ic0 + r["n_cols"]) * 8],
                            num_idxs=L, num_idxs_reg=L, elem_size=in_c,
                            single_packet=False, queue_num=r["queue"],
                        )
                    else:
                        row0 = meta["core"] * n_per + r["row0"]
                        full = r["rows"] // 128
                        rem = r["rows"] - full * 128
                        if full:
                            nc.sync.dma_start(
                                out=rt[:, :full, :],
                                in_=x[row0:row0 + full * 128, :].rearrange(
                                    "(c p) d -> p c d", p=128),
                            )
                        if rem:
                            nc.sync.dma_start(
                                out=rt[:rem, full, :],
                                in_=x[row0 + full * 128:row0 + r["rows"], :],
                            )
                    run_tiles.append(rt)

                for blk in win["blocks"]:
                    rt = run_tiles[blk["run_i"]]
                    lhsT = rt[:, blk["bcol"], :]
                    for (g0, ng, mg_i, j0) in blk["chunks"]:
                        rhs = m_tiles[mg_i][:, j0:j0 + ng, :]
                        nc.tensor.matmul(
                            out=psum_w[:, g0 * WS:(g0 + ng) * WS],
                            lhsT=lhsT, rhs=rhs,
                            start=False, stop=False, skip_group_check=True,
                        )

                aggT = apool.tile([P, WSLOTS], F32, tag="aggT")
                nc.scalar.activation(aggT[:], psum_w[:], mybir.ActivationFunctionType.Copy)

                psum_o = popool.tile([P, WSLOTS // P, HID], F32, tag="po")
                n_chunk = min(WSLOTS // P, (n_per - w * WSLOTS + P - 1) // P)
                b_b = AP(b_t[:].tensor, b_t[:].offset,
                         [b_t[:].ap[0], [0, n_chunk], [1, HID]])
                nc.scalar.activation(psum_o[:, :n_chunk, :], b_b,
                                     mybir.ActivationFunctionType.Copy)
                for s in range(n_chunk):
                    nc.tensor.matmul(out=psum_o[:, s, :],
                                     lhsT=aggT[:, s * P:(s + 1) * P], rhs=wmat[:],
                                     start=False, stop=False, skip_group_check=True)
                # prelu(v) = relu(v) - pw * relu(-v)
                r_t = fpool.tile([P, WSLOTS // P, HID], F32, tag="r")
                nc.scalar.activation(r_t[:, :n_chunk, :], psum_o[:, :n_chunk, :],
                                     mybir.ActivationFunctionType.Relu)
                nr_t = fpool.tile([P, WSLOTS // P, HID], F32, tag="nr")
                nc.scalar.activation(nr_t[:, :n_chunk, :], psum_o[:, :n_chunk, :],
                                     mybir.ActivationFunctionType.Relu, scale=-1.0)
                pw_b = AP(pw_t[:].tensor, pw_t[:].offset,
                          [pw_t[:].ap[0], [0, n_chunk], [1, HID]])
                nm_t = fpool.tile([P, WSLOTS // P, HID], F32, tag="nm")
                nc.vector.tensor_tensor(out=nm_t[:, :n_chunk, :], in0=nr_t[:, :n_chunk, :],
                                        in1=pw_b, op=mybir.AluOpType.mult)
                nc.vector.tensor_tensor(out=nm_t[:, :n_chunk, :], in0=r_t[:, :n_chunk, :],
                                        in1=nm_t[:, :n_chunk, :], op=mybir.AluOpType.subtract)
                for s in range(n_chunk):
                    row0 = w * WSLOTS + s * P
                    nrow = min(P, n_per - row0)
                    nc.sync.dma_start(out=out[row0:row0 + nrow, :],
                                      in_=nm_t[:nrow, s, :])
    nc.compile()
    return nc


def build_all(edge_index, n_nodes, n_cores=8):
    src = np.asarray(edge_index[0], dtype=np.int64)
    dst = np.asarray(edge_index[1], dtype=np.int64)
    deg = np.bincount(dst, minlength=n_nodes).astype(np.int64) + 1
    metas = []
    for c in range(n_cores):
        meta = build_core_metadata(src, dst, deg, n_nodes, n_cores, c)
        meta["n_nodes"] = n_nodes
        metas.append(meta)
    with ThreadPoolExecutor(max_workers=n_cores) as ex:
        ncs = list(ex.map(build_core_kernel, metas))
    return metas, ncs


def make_in_map(meta, x, W, b, prelu_w):
    b_bc = np.tile(np.asarray(b, np.float32)[None, :], (P, 1)).copy()
    pw_bc = np.tile(np.asarray(prelu_w, np.float32)[None, :], (P, 1)).copy()
    gcols = max(meta["gtot"] // 128, 1)
    n_pieces = max(meta["n_pieces"], 1)
    idx = meta["idx"]
    if idx.shape[1] < gcols * 8:
        idx = np.zeros((P, gcols * 8), np.int16)
        idx[:, :meta["idx"].shape[1]] = meta["idx"]
    dstst = meta["dstst"]
    dppc = meta["dppc"]
    if dstst.shape[1] < n_pieces:
        t = np.full((P, n_pieces), -1.0, np.float32); t[:, :dstst.shape[1]] = dstst; dstst = t
        t = np.ones((P, n_pieces), np.float32); t[:, :dppc.shape[1]] = dppc; dppc = t
    return {
        "x": np.ascontiguousarray(np.asarray(x, np.float32).astype(np.float16)),
        "w_in": np.ascontiguousarray(np.asarray(W, np.float32)),
        "b_bc": b_bc, "pw_bc": pw_bc,
        "idx": np.ascontiguousarray(idx),
        "dstst": np.ascontiguousarray(dstst.astype(np.float16)),
        "dppc": np.ascontiguousarray(dppc),
    }

_CACHE = {}


def _run_one(nc, in_map, dev):
    import jax

    last = None
    for _ in range(3):  # retry transient device faults
        try:
            with jax.default_device(dev):
                r = run_bass_kernel_spmd(nc, [in_map], core_ids=[0])
            return r.results[0]["out"]
        except Exception as e:  # noqa: BLE001
            last = e
    raise last


def kernel(x, edge_index, W, b, prelu_w):
    import jax

    x = np.asarray(x)
    edge_index = np.asarray(edge_index)
    W = np.asarray(W)
    b = np.asarray(b)
    prelu_w = np.asarray(prelu_w)
    n_nodes = x.shape[0]
    n_cores = 8

    key = hash((edge_index.tobytes(), n_nodes))
    if _CACHE.get("key") != key:
        metas, ncs = build_all(edge_index, n_nodes, n_cores)
        _CACHE.update(key=key, metas=metas, ncs=ncs)
    metas, ncs = _CACHE["metas"], _CACHE["ncs"]

    maps = [make_in_map(m, x, W, b, prelu_w) for m in metas]
    devs = jax.devices()[:n_cores]
    outs = [_run_one(ncs[c], maps[c], devs[c]) for c in range(n_cores)]
    return np.concatenate(outs, axis=0).astype(np.float32)

